# Initial kernel scaffold
#
"""Multi-head attention (B=4, S=2048, D=1024, H=16) on 8 Trainium2 NeuronCores.

Sharding: 4-way data-parallel over batch x 2-way tensor-parallel over heads
(Megatron-style).  Core c handles batch c//2 and head-group c%2 (8 of 16
heads).  Each core computes qkv for its 512 q/k/v channels, attention for its
8 heads, and a row-parallel partial projection [S, D].  The host sums the two
partial outputs per batch and adds b_proj.

Per-core kernel strategy:
  - x^T tiles produced on-chip via PE transpose (DMA transpose unsupported
    for fp32).
  - Q^T, K^T computed channel-major [ch, s] from w-chunks (lhsT) x x^T (rhs);
    V computed natural [s, ch] from x^T-chunks (lhsT) x w_v (rhs).
  - Scores computed transposed: S^T[kj, qi] = K Q^T so softmax normalization
    folds into the PV matmul: lhsT = [V | ones] yields attn^T[ch, qi] plus
    row-sums in one accumulated matmul chain (one PSUM start per bank).
  - All big matmuls in float32r (TF32-like, full PE rate at N=512, rms err
    ~1.5e-4); operands rounded via compute-engine copies as walrus requires.
  - exp on ScalarE with the 1/sqrt(hd) scale folded in; normalization via
    a K=1 PE broadcast of the row-sums + approx-reciprocal + multiply at
    PSUM eviction, deferred one block so it never stalls the QK->exp chain.
"""

import sys
from contextlib import ExitStack

for _p in ("/opt/trn_rl_repo", "/root/.axon_site/_ro/trn_rl_repo"):
    if _p not in sys.path:
        sys.path.insert(0, _p)

import numpy as np

import concourse.bass as bass  # noqa: F401
import concourse.mybir as mybir
import concourse.tile as tile
from concourse import bacc
from concourse.bass_utils import run_bass_kernel_spmd
from concourse.masks import make_identity

F32 = mybir.dt.float32
F32R = mybir.dt.float32r
EXP = mybir.ActivationFunctionType.Exp

N_CORES = 8
FULL_B, FULL_S, FULL_D, FULL_H = 4, 2048, 1024, 16
HEAD_DIM = 64


def build_core_program(S=FULL_S, D=FULL_D, HL=FULL_H // 2, hd=HEAD_DIM):
    """Build the single-core Bass program (runs SPMD on all 8 cores with
    per-core input shards)."""
    CH = HL * hd            # local q (= k = v) channels (512)
    DC = D // 128           # d-chunks (qkv contraction)
    CCQ = CH // 128         # ch-chunks for Q (and K)
    CCA = CH // 128         # ch-chunks of attn output (proj contraction)
    SC = S // 128           # 128-row s-chunks (also kj-chunks)
    SBLK = S // 512         # 512-row s-blocks in the qkv phase
    QBS = min(1024, S)      # qi block size in the attention phase
    QB = S // QBS
    NB = QBS // 512
    scale = float(hd) ** -0.5

    nc = bacc.Bacc("TRN2", target_bir_lowering=False, debug=False,
                   num_devices=N_CORES)

    x_ap = nc.dram_tensor("x", [S, D], F32, kind="ExternalInput").ap()
    wqkv_ap = nc.dram_tensor("w_qkv", [D, 3 * CH], F32, kind="ExternalInput").ap()
    bqkv_ap = nc.dram_tensor("b_qkv", [3 * CH], F32, kind="ExternalInput").ap()
    wproj_ap = nc.dram_tensor("w_proj", [CH, D], F32, kind="ExternalInput").ap()
    out_ap = nc.dram_tensor("out", [S, D], F32, kind="ExternalOutput").ap()

    with tile.TileContext(nc) as tc, ExitStack() as es:
        constp = es.enter_context(tc.tile_pool(name="const", bufs=1))
        qk_es = es.enter_context(ExitStack())
        qkp = qk_es.enter_context(tc.tile_pool(name="qk", bufs=1))

        ident = constp.tile([128, 128], F32)
        make_identity(nc, ident)
        bias_qk = constp.tile([128, 2 * CCQ], F32)
        nc.sync.dma_start(bias_qk[:],
                          bqkv_ap[0:2 * CH].rearrange("(c p) -> p c", p=128))
        bv_row = constp.tile([1, CH], F32)
        nc.sync.dma_start(bv_row[:],
                          bqkv_ap[2 * CH:3 * CH].rearrange("(a b) -> a b", a=1))
        bv_bc = constp.tile([128, CH], F32)
        nc.gpsimd.partition_broadcast(bv_bc[:], bv_row[0:1, :])
        ones_f = constp.tile([128, SC * HL], F32)
        nc.vector.memset(ones_f[:], 1.0)
        ones65_f = constp.tile([65, 64], F32)
        nc.vector.memset(ones65_f[:], 1.0)
        ones65 = constp.tile([65, 64], F32R)
        nc.vector.tensor_copy(ones65[:], ones65_f[:])

        # persistent activations (released after phase 2)
        qT = qkp.tile([128, CCQ, S], F32R)            # [ch, s]
        kT = qkp.tile([128, CCQ, S], F32R)
        vp = qkp.tile([128, SC, HL, hd + 2], F32R)    # [s | kj, head, V|1|pad]
        nc.vector.tensor_copy(vp[:, :, :, hd],
                              ones_f[:].rearrange("p (a b) -> p a b", b=HL))

        # ---------------- phase 1: weights, x^T, qkv ----------------
        with ExitStack() as p1:
            wqp = p1.enter_context(tc.tile_pool(name="wq", bufs=1))
            stagep = p1.enter_context(tc.tile_pool(name="stage", bufs=1))
            ps1 = p1.enter_context(tc.tile_pool(name="ps1", bufs=1, space="PSUM"))

            wq_r = wqp.tile([128, DC, 3 * CH], F32R)
            HW_ = 3 * CH // 2
            for dc in range(DC):
                for half in range(2):
                    wq_f = wqp.tile([128, HW_], F32, tag="wq_st", bufs=2)
                    nc.sync.dma_start(
                        wq_f[:], wqkv_ap[dc * 128:(dc + 1) * 128,
                                         half * HW_:(half + 1) * HW_])
                    nc.vector.tensor_copy(
                        wq_r[:, dc, half * HW_:(half + 1) * HW_], wq_f[:])

            for sb_i in range(SBLK):
                xs_tiles = []
                for i in range(4):
                    xsi = stagep.tile([128, D], F32, tag="xs", bufs=4)
                    nc.sync.dma_start(
                        xsi[:], x_ap[sb_i * 512 + i * 128:
                                     sb_i * 512 + (i + 1) * 128, :])
                    xs_tiles.append(xsi)
                xT = stagep.tile([128, DC, 512], F32R, tag="xT", bufs=2)
                for dc in range(DC):
                    tp = ps1.tile([128, 512], F32, tag="tp", bufs=3)
                    for i in range(4):
                        nc.tensor.transpose(tp[:, i * 128:(i + 1) * 128],
                                            xs_tiles[i][:, dc * 128:(dc + 1) * 128],
                                            ident[:])
                    nc.vector.tensor_copy(xT[:, dc, :], tp[:])
                # Q^T / K^T chunks: out [ch 128, s 512]
                for cc in range(2 * CCQ):
                    qp = ps1.tile([128, 512], F32, tag="qkv", bufs=4)
                    for dc in range(DC):
                        nc.tensor.matmul(qp[:],
                                         wq_r[:, dc, cc * 128:(cc + 1) * 128],
                                         xT[:, dc, :],
                                         start=(dc == 0), stop=(dc == DC - 1))
                    dst = qT if cc < CCQ else kT
                    cc_l = cc if cc < CCQ else cc - CCQ
                    nc.vector.tensor_scalar_add(
                        dst[:, cc_l, sb_i * 512:(sb_i + 1) * 512], qp[:],
                        bias_qk[:, cc:cc + 1])
                # V natural: out [s 128, ch 512]
                for si in range(4):
                    vps = ps1.tile([128, CH], F32, tag="qkv", bufs=4)
                    for dc in range(DC):
                        nc.tensor.matmul(vps[:],
                                         xT[:, dc, si * 128:(si + 1) * 128],
                                         wq_r[:, dc, 2 * CH:3 * CH],
                                         start=(dc == 0), stop=(dc == DC - 1))
                    sc_g = sb_i * 4 + si
                    nc.vector.tensor_add(
                        vp[:, sc_g, :, 0:hd],
                        vps[:].rearrange("p (h e) -> p h e", e=hd),
                        bv_bc[:].rearrange("p (h e) -> p h e", e=hd))

        # ---------------- phase 2: attention ----------------
        attn_es = es.enter_context(ExitStack())
        attnp = attn_es.enter_context(
            tc.tile_pool(name="attn", bufs=1, side="right"))
        attn_r = attnp.tile([128, CCA, S], F32R)      # normalized attn^T (f32r)
        attn_t_odd = attnp.tile([128, CCA, S], F32)   # staging for odd heads

        with ExitStack() as p2:
            workp = p2.enter_context(tc.tile_pool(name="w2", bufs=1))
            ps2 = p2.enter_context(tc.tile_pool(name="ps2", bufs=1, space="PSUM"))

            def emit_norm(cc, base, qb, attn_ps):
                # normalize columns by 1/rowsum, store into attn_r.  Sums sit
                # on psum row 64; PE-broadcast to rows 0..63 (K=1 matmul),
                # approx-reciprocal, multiply.  Emitted one block late so the
                # PE never stalls the QK->exp chain on this detour.
                sums_sb = workp.tile([65, QBS], F32R, tag="asb", bufs=2)
                nc.vector.tensor_copy(sums_sb[64:65, :], attn_ps[64:65, :])
                bc = ps2.tile([64, QBS], F32, tag="sc", bufs=2)
                for nb in range(NB):
                    nc.tensor.matmul(bc[:, nb * 512:(nb + 1) * 512],
                                     ones65[64:65, 0:64],
                                     sums_sb[64:65, nb * 512:(nb + 1) * 512],
                                     start=True, stop=True,
                                     tile_position=(64, 0))
                recip = workp.tile([64, QBS], F32, tag="norm", bufs=2)
                nc.vector.reciprocal_approx_fast(recip[:], bc[:])
                if base == 0:
                    nc.vector.tensor_mul(
                        attn_r[0:64, cc, qb * QBS:(qb + 1) * QBS],
                        attn_ps[0:64, :], recip[:])
                else:
                    asb = workp.tile([64, QBS], F32, tag="asb", bufs=2)
                    nc.vector.tensor_mul(asb[:], attn_ps[0:64, :], recip[:])
                    nc.sync.dma_start(
                        attn_t_odd[64:128, cc, qb * QBS:(qb + 1) * QBS],
                        asb[:])
                    if qb == QB - 1:
                        nc.vector.tensor_copy(attn_r[64:128, cc, :],
                                              attn_t_odd[64:128, cc, :])

            pending = None
            for h in range(HL):
                cc, base = h // 2, (h % 2) * 64
                for qb in range(QB):
                    attn_ps = ps2.tile([65, QBS], F32, tag="attn", bufs=2)
                    for kj in range(SC):
                        if kj == 1 and pending is not None:
                            emit_norm(*pending)
                            pending = None
                        sc_ps = ps2.tile([128, QBS], F32, tag="sc", bufs=2)
                        for nb in range(NB):
                            nc.tensor.matmul(
                                sc_ps[:, nb * 512:(nb + 1) * 512],
                                kT[base:base + 64, cc, kj * 128:(kj + 1) * 128],
                                qT[base:base + 64, cc,
                                   qb * QBS + nb * 512:qb * QBS + (nb + 1) * 512],
                                start=True, stop=True,
                                tile_position=(base, 0))
                        pt = workp.tile([128, QBS], F32R, tag="pt", bufs=2)
                        nc.scalar.activation(pt[:], sc_ps[:], EXP, scale=scale)
                        for nb in range(NB):
                            nc.tensor.matmul(
                                attn_ps[:, nb * 512:(nb + 1) * 512],
                                vp[:, kj, h, 0:hd + 1],
                                pt[:, nb * 512:(nb + 1) * 512],
                                start=(kj == 0), stop=(kj == SC - 1))
                    pending = (cc, base, qb, attn_ps)
            emit_norm(*pending)

        qk_es.close()  # free qT/kT/vp

        # ---------------- phase 3: projection ----------------
        with ExitStack() as p3:
            w3 = p3.enter_context(tc.tile_pool(name="w3", bufs=1))
            ps3 = p3.enter_context(tc.tile_pool(name="ps3", bufs=1, space="PSUM"))


            wp_r = w3.tile([128, CCA, D], F32R)
            for cc4 in range(CCA):
                wp_f = w3.tile([128, D], F32, tag="wp_st", bufs=2)
                nc.sync.dma_start(wp_f[:],
                                  wproj_ap[cc4 * 128:(cc4 + 1) * 128, :])
                nc.vector.tensor_copy(wp_r[:, cc4, :], wp_f[:])
            for sc_i in range(SC):
                pp = ps3.tile([128, D], F32, tag="proj", bufs=3)
                for cc4 in range(CCA):
                    for nh in range(D // 512):
                        nc.tensor.matmul(
                            pp[:, nh * 512:(nh + 1) * 512],
                            attn_r[:, cc4, sc_i * 128:(sc_i + 1) * 128],
                            wp_r[:, cc4, nh * 512:(nh + 1) * 512],
                            start=(cc4 == 0), stop=(cc4 == CCA - 1))
                osb = w3.tile([128, D], F32, tag="osb", bufs=3)
                nc.vector.tensor_copy(osb[:], pp[:])
                nc.sync.dma_start(out_ap[sc_i * 128:(sc_i + 1) * 128, :], osb[:])

    nc.compile()
    return nc


def shard_inputs(x, w_qkv, b_qkv, w_proj):
    """Full inputs -> per-core input maps. Core c: batch c//2, head-group c%2."""
    B, S, D = x.shape
    CH = D // 2
    in_maps = []
    for c in range(N_CORES):
        b, g = c // 2, c % 2
        sl = slice(g * CH, (g + 1) * CH)
        w_s = np.concatenate(
            [w_qkv[:, 0 * D + g * CH:0 * D + (g + 1) * CH],
             w_qkv[:, 1 * D + g * CH:1 * D + (g + 1) * CH],
             w_qkv[:, 2 * D + g * CH:2 * D + (g + 1) * CH]], axis=1)
        b_s = np.concatenate(
            [b_qkv[0 * D + g * CH:0 * D + (g + 1) * CH],
             b_qkv[1 * D + g * CH:1 * D + (g + 1) * CH],
             b_qkv[2 * D + g * CH:2 * D + (g + 1) * CH]], axis=0)
        in_maps.append({
            "x": np.ascontiguousarray(x[b]),
            "w_qkv": np.ascontiguousarray(w_s),
            "b_qkv": np.ascontiguousarray(b_s),
            "w_proj": np.ascontiguousarray(w_proj[sl, :]),
        })
    return in_maps


_PROGRAM = None


def _get_program():
    global _PROGRAM
    if _PROGRAM is None:
        _PROGRAM = build_core_program()
    return _PROGRAM


def run_sharded(nc, in_maps, **kw):
    """run_bass_kernel_spmd with retries: the first execution on a freshly
    attached device occasionally dies with NRT_EXEC_UNIT_UNRECOVERABLE."""
    last = None
    for _ in range(3):
        try:
            return run_bass_kernel_spmd(nc, in_maps,
                                        core_ids=list(range(N_CORES)), **kw)
        except Exception as e:  # noqa: BLE001
            last = e
    raise last


def kernel(x, w_qkv, b_qkv, w_proj, b_proj):
    x = np.asarray(x, dtype=np.float32)
    w_qkv = np.asarray(w_qkv, dtype=np.float32)
    b_qkv = np.asarray(b_qkv, dtype=np.float32)
    w_proj = np.asarray(w_proj, dtype=np.float32)
    b_proj = np.asarray(b_proj, dtype=np.float32)

    nc = _get_program()
    in_maps = shard_inputs(x, w_qkv, b_qkv, w_proj)
    res = run_sharded(nc, in_maps)

    B, S, D = x.shape
    out = np.empty((B, S, D), dtype=np.float32)
    for b in range(B):
        out[b] = res.results[2 * b]["out"] + res.results[2 * b + 1]["out"] + b_proj
    return out



# revision 1
# speedup vs baseline: 2.8453x; 2.8453x over previous
"""Multi-head attention (B=4, S=2048, D=1024, H=16) on 8 Trainium2 NeuronCores.

Sharding: 4-way data-parallel over batch x 2-way tensor-parallel over heads
(Megatron-style).  Core c handles batch c//2 and head-group c%2 (8 of 16
heads).  Each core computes qkv for its 512 q/k/v channels, attention for its
8 heads, and a row-parallel partial projection [S, D].  The host sums the two
partial outputs per batch and adds b_proj.

Per-core kernel strategy:
  - x^T tiles produced on-chip via PE transpose (DMA transpose unsupported
    for fp32).
  - Q^T, K^T computed channel-major [ch, s] from w-chunks (lhsT) x x^T (rhs);
    V computed natural [s, ch] from x^T-chunks (lhsT) x w_v (rhs).
  - Scores computed transposed: S^T[kj, qi] = K Q^T so softmax normalization
    folds into the PV matmul: lhsT = [V | ones] yields attn^T[ch, qi] plus
    row-sums in one accumulated matmul chain (one PSUM start per bank).
  - All big matmuls in float32r (TF32-like, full PE rate at N=512, rms err
    ~1.5e-4); operands rounded via compute-engine copies as walrus requires.
  - exp on ScalarE with the 1/sqrt(hd) scale folded in; normalization via
    a K=1 PE broadcast of the row-sums + approx-reciprocal + multiply at
    PSUM eviction, deferred one block so it never stalls the QK->exp chain.
"""

import sys
from contextlib import ExitStack

for _p in ("/opt/trn_rl_repo", "/root/.axon_site/_ro/trn_rl_repo"):
    if _p not in sys.path:
        sys.path.insert(0, _p)

import numpy as np

import concourse.bass as bass  # noqa: F401
import concourse.mybir as mybir
import concourse.tile as tile
from concourse import bacc
from concourse.bass_utils import run_bass_kernel_spmd
from concourse.masks import make_identity

F32 = mybir.dt.float32
F32R = mybir.dt.float32r
EXP = mybir.ActivationFunctionType.Exp

N_CORES = 8
FULL_B, FULL_S, FULL_D, FULL_H = 4, 2048, 1024, 16
HEAD_DIM = 64


def build_core_program(S=FULL_S, D=FULL_D, HL=FULL_H // 2, hd=HEAD_DIM):
    """Build the single-core Bass program (runs SPMD on all 8 cores with
    per-core input shards)."""
    CH = HL * hd            # local q (= k = v) channels (512)
    DC = D // 128           # d-chunks (qkv contraction)
    CCQ = CH // 128         # ch-chunks for Q (and K)
    CCA = CH // 128         # ch-chunks of attn output (proj contraction)
    SC = S // 128           # 128-row s-chunks (also kj-chunks)
    SBLK = S // 512         # 512-row s-blocks in the qkv phase
    QBS = min(1024, S)      # qi block size in the attention phase
    QB = S // QBS
    NB = QBS // 512
    scale = float(hd) ** -0.5

    nc = bacc.Bacc("TRN2", target_bir_lowering=False, debug=False,
                   num_devices=N_CORES)

    x_ap = nc.dram_tensor("x", [S, D], F32, kind="ExternalInput").ap()
    wqkv_ap = nc.dram_tensor("w_qkv", [D, 3 * CH], F32, kind="ExternalInput").ap()
    bqkv_ap = nc.dram_tensor("b_qkv", [3 * CH], F32, kind="ExternalInput").ap()
    wproj_ap = nc.dram_tensor("w_proj", [CH, D], F32, kind="ExternalInput").ap()
    out_ap = nc.dram_tensor("out", [S, D], F32, kind="ExternalOutput").ap()

    with tile.TileContext(nc) as tc, ExitStack() as es:
        constp = es.enter_context(tc.tile_pool(name="const", bufs=1))
        qk_es = es.enter_context(ExitStack())
        qkp = qk_es.enter_context(tc.tile_pool(name="qk", bufs=1))

        ident = constp.tile([128, 128], F32)
        make_identity(nc, ident)
        bias_qk = constp.tile([128, 2 * CCQ], F32)
        nc.sync.dma_start(bias_qk[:],
                          bqkv_ap[0:2 * CH].rearrange("(c p) -> p c", p=128))
        bv_row = constp.tile([1, CH], F32)
        nc.sync.dma_start(bv_row[:],
                          bqkv_ap[2 * CH:3 * CH].rearrange("(a b) -> a b", a=1))
        bv_bc = constp.tile([128, CH], F32)
        nc.gpsimd.partition_broadcast(bv_bc[:], bv_row[0:1, :])
        ones_f = constp.tile([128, SC * HL], F32)
        nc.vector.memset(ones_f[:], 1.0)
        ones65_f = constp.tile([65, 64], F32)
        nc.vector.memset(ones65_f[:], 1.0)
        ones65 = constp.tile([65, 64], F32R)
        nc.vector.tensor_copy(ones65[:], ones65_f[:])

        # persistent activations (released after phase 2)
        qT = qkp.tile([128, CCQ, S], F32R)            # [ch, s]
        kT = qkp.tile([128, CCQ, S], F32R)
        vp = qkp.tile([128, SC, HL, hd + 2], F32R)    # [s | kj, head, V|1|pad]
        nc.vector.tensor_copy(vp[:, :, :, hd],
                              ones_f[:].rearrange("p (a b) -> p a b", b=HL))

        # ---------------- phase 1: weights, x^T, qkv ----------------
        with ExitStack() as p1:
            wqp = p1.enter_context(tc.tile_pool(name="wq", bufs=1))
            stagep = p1.enter_context(tc.tile_pool(name="stage", bufs=1))
            ps1 = p1.enter_context(tc.tile_pool(name="ps1", bufs=1, space="PSUM"))

            wq_r = wqp.tile([128, DC, 3 * CH], F32R)
            HW_ = 3 * CH // 2
            for dc in range(DC):
                for half in range(2):
                    wq_f = wqp.tile([128, HW_], F32, tag="wq_st", bufs=2)
                    nc.sync.dma_start(
                        wq_f[:], wqkv_ap[dc * 128:(dc + 1) * 128,
                                         half * HW_:(half + 1) * HW_])
                    nc.vector.tensor_copy(
                        wq_r[:, dc, half * HW_:(half + 1) * HW_], wq_f[:])

            for sb_i in range(SBLK):
                xs_tiles = []
                for i in range(4):
                    xsi = stagep.tile([128, D], F32, tag="xs", bufs=4)
                    nc.sync.dma_start(
                        xsi[:], x_ap[sb_i * 512 + i * 128:
                                     sb_i * 512 + (i + 1) * 128, :])
                    xs_tiles.append(xsi)
                xT = stagep.tile([128, DC, 512], F32R, tag="xT", bufs=2)
                for dc in range(DC):
                    tp = ps1.tile([128, 512], F32, tag="tp", bufs=3)
                    for i in range(4):
                        nc.tensor.transpose(tp[:, i * 128:(i + 1) * 128],
                                            xs_tiles[i][:, dc * 128:(dc + 1) * 128],
                                            ident[:])
                    nc.vector.tensor_copy(xT[:, dc, :], tp[:])
                # Q^T / K^T chunks: out [ch 128, s 512]
                for cc in range(2 * CCQ):
                    qp = ps1.tile([128, 512], F32, tag="qkv", bufs=4)
                    for dc in range(DC):
                        nc.tensor.matmul(qp[:],
                                         wq_r[:, dc, cc * 128:(cc + 1) * 128],
                                         xT[:, dc, :],
                                         start=(dc == 0), stop=(dc == DC - 1))
                    dst = qT if cc < CCQ else kT
                    cc_l = cc if cc < CCQ else cc - CCQ
                    nc.vector.tensor_scalar_add(
                        dst[:, cc_l, sb_i * 512:(sb_i + 1) * 512], qp[:],
                        bias_qk[:, cc:cc + 1])
                # V natural: out [s 128, ch 512]
                for si in range(4):
                    vps = ps1.tile([128, CH], F32, tag="qkv", bufs=4)
                    for dc in range(DC):
                        nc.tensor.matmul(vps[:],
                                         xT[:, dc, si * 128:(si + 1) * 128],
                                         wq_r[:, dc, 2 * CH:3 * CH],
                                         start=(dc == 0), stop=(dc == DC - 1))
                    sc_g = sb_i * 4 + si
                    nc.vector.tensor_add(
                        vp[:, sc_g, :, 0:hd],
                        vps[:].rearrange("p (h e) -> p h e", e=hd),
                        bv_bc[:].rearrange("p (h e) -> p h e", e=hd))

        # ---------------- phase 2: attention ----------------
        attn_es = es.enter_context(ExitStack())
        attnp = attn_es.enter_context(
            tc.tile_pool(name="attn", bufs=1, side="right"))
        attn_r = attnp.tile([128, CCA, S], F32R)      # normalized attn^T (f32r)
        attn_t_odd = attnp.tile([128, CCA, S], F32)   # staging for odd heads

        with ExitStack() as p2:
            workp = p2.enter_context(tc.tile_pool(name="w2", bufs=1))
            ps2 = p2.enter_context(tc.tile_pool(name="ps2", bufs=1, space="PSUM"))

            def emit_norm(cc, base, qb, attn_ps):
                # normalize columns by 1/rowsum, store into attn_r.  Sums sit
                # on psum row 64; PE-broadcast to rows 0..63 (K=1 matmul),
                # approx-reciprocal, multiply.  Emitted one block late so the
                # PE never stalls the QK->exp chain on this detour.
                sums_sb = workp.tile([65, QBS], F32R, tag="asb", bufs=2)
                nc.vector.tensor_copy(sums_sb[64:65, :], attn_ps[64:65, :])
                bc = ps2.tile([64, QBS], F32, tag="sc", bufs=2)
                for nb in range(NB):
                    nc.tensor.matmul(bc[:, nb * 512:(nb + 1) * 512],
                                     ones65[64:65, 0:64],
                                     sums_sb[64:65, nb * 512:(nb + 1) * 512],
                                     start=True, stop=True,
                                     tile_position=(64, 0))
                recip = workp.tile([64, QBS], F32, tag="norm", bufs=2)
                nc.vector.reciprocal_approx_fast(recip[:], bc[:])
                if base == 0:
                    nc.vector.tensor_mul(
                        attn_r[0:64, cc, qb * QBS:(qb + 1) * QBS],
                        attn_ps[0:64, :], recip[:])
                else:
                    asb = workp.tile([64, QBS], F32, tag="asb", bufs=2)
                    nc.vector.tensor_mul(asb[:], attn_ps[0:64, :], recip[:])
                    nc.sync.dma_start(
                        attn_t_odd[64:128, cc, qb * QBS:(qb + 1) * QBS],
                        asb[:])
                    if qb == QB - 1:
                        nc.vector.tensor_copy(attn_r[64:128, cc, :],
                                              attn_t_odd[64:128, cc, :])

            pending = None
            for h in range(HL):
                cc, base = h // 2, (h % 2) * 64
                for qb in range(QB):
                    attn_ps = ps2.tile([65, QBS], F32, tag="attn", bufs=2)
                    for kj in range(SC):
                        if kj == 1 and pending is not None:
                            emit_norm(*pending)
                            pending = None
                        sc_ps = ps2.tile([128, QBS], F32, tag="sc", bufs=2)
                        for nb in range(NB):
                            nc.tensor.matmul(
                                sc_ps[:, nb * 512:(nb + 1) * 512],
                                kT[base:base + 64, cc, kj * 128:(kj + 1) * 128],
                                qT[base:base + 64, cc,
                                   qb * QBS + nb * 512:qb * QBS + (nb + 1) * 512],
                                start=True, stop=True,
                                tile_position=(base, 0))
                        pt = workp.tile([128, QBS], F32R, tag="pt", bufs=2)
                        nc.scalar.activation(pt[:], sc_ps[:], EXP, scale=scale)
                        for nb in range(NB):
                            nc.tensor.matmul(
                                attn_ps[:, nb * 512:(nb + 1) * 512],
                                vp[:, kj, h, 0:hd + 1],
                                pt[:, nb * 512:(nb + 1) * 512],
                                start=(kj == 0), stop=(kj == SC - 1))
                    pending = (cc, base, qb, attn_ps)
            emit_norm(*pending)

        qk_es.close()  # free qT/kT/vp

        # ---------------- phase 3: projection ----------------
        with ExitStack() as p3:
            w3 = p3.enter_context(tc.tile_pool(name="w3", bufs=1))
            ps3 = p3.enter_context(tc.tile_pool(name="ps3", bufs=1, space="PSUM"))


            wp_r = w3.tile([128, CCA, D], F32R)
            for cc4 in range(CCA):
                wp_f = w3.tile([128, D], F32, tag="wp_st", bufs=2)
                nc.sync.dma_start(wp_f[:],
                                  wproj_ap[cc4 * 128:(cc4 + 1) * 128, :])
                nc.vector.tensor_copy(wp_r[:, cc4, :], wp_f[:])
            for sc_i in range(SC):
                pp = ps3.tile([128, D], F32, tag="proj", bufs=3)
                for cc4 in range(CCA):
                    for nh in range(D // 512):
                        nc.tensor.matmul(
                            pp[:, nh * 512:(nh + 1) * 512],
                            attn_r[:, cc4, sc_i * 128:(sc_i + 1) * 128],
                            wp_r[:, cc4, nh * 512:(nh + 1) * 512],
                            start=(cc4 == 0), stop=(cc4 == CCA - 1))
                osb = w3.tile([128, D], F32, tag="osb", bufs=3)
                nc.vector.tensor_copy(osb[:], pp[:])
                nc.sync.dma_start(out_ap[sc_i * 128:(sc_i + 1) * 128, :], osb[:])

    nc.compile()
    return nc


def shard_inputs(x, w_qkv, b_qkv, w_proj):
    """Full inputs -> per-core input maps. Core c: batch c//2, head-group c%2."""
    B, S, D = x.shape
    CH = D // 2
    in_maps = []
    for c in range(N_CORES):
        b, g = c // 2, c % 2
        sl = slice(g * CH, (g + 1) * CH)
        w_s = np.concatenate(
            [w_qkv[:, 0 * D + g * CH:0 * D + (g + 1) * CH],
             w_qkv[:, 1 * D + g * CH:1 * D + (g + 1) * CH],
             w_qkv[:, 2 * D + g * CH:2 * D + (g + 1) * CH]], axis=1)
        b_s = np.concatenate(
            [b_qkv[0 * D + g * CH:0 * D + (g + 1) * CH],
             b_qkv[1 * D + g * CH:1 * D + (g + 1) * CH],
             b_qkv[2 * D + g * CH:2 * D + (g + 1) * CH]], axis=0)
        in_maps.append({
            "x": np.ascontiguousarray(x[b]),
            "w_qkv": np.ascontiguousarray(w_s),
            "b_qkv": np.ascontiguousarray(b_s),
            "w_proj": np.ascontiguousarray(w_proj[sl, :]),
        })
    return in_maps


_PROGRAM = None


def _get_program():
    global _PROGRAM
    if _PROGRAM is None:
        _PROGRAM = build_core_program()
    return _PROGRAM


def run_sharded(nc, in_maps, **kw):
    """run_bass_kernel_spmd with retries: the first execution on a freshly
    attached device occasionally dies with NRT_EXEC_UNIT_UNRECOVERABLE."""
    last = None
    for _ in range(3):
        try:
            return run_bass_kernel_spmd(nc, in_maps,
                                        core_ids=list(range(N_CORES)), **kw)
        except Exception as e:  # noqa: BLE001
            last = e
    raise last


def kernel(x, w_qkv, b_qkv, w_proj, b_proj):
    x = np.asarray(x, dtype=np.float32)
    w_qkv = np.asarray(w_qkv, dtype=np.float32)
    b_qkv = np.asarray(b_qkv, dtype=np.float32)
    w_proj = np.asarray(w_proj, dtype=np.float32)
    b_proj = np.asarray(b_proj, dtype=np.float32)

    nc = _get_program()
    in_maps = shard_inputs(x, w_qkv, b_qkv, w_proj)
    res = run_sharded(nc, in_maps)

    B, S, D = x.shape
    out = np.empty((B, S, D), dtype=np.float32)
    for b in range(B):
        out[b] = res.results[2 * b]["out"] + res.results[2 * b + 1]["out"] + b_proj
    return out

